# revision 4
# baseline (speedup 1.0000x reference)
"""Trainium2 Bass kernel for nn_DeepESNGatedGRU — v2 (fused-layer pipeline).

Data-parallel over batch (8 cores, B_local=8). Everything on-chip in a
transposed layout (feature dim on SBUF partitions, (batch,time) on free).

v2 over the v1 baseline:
  * The two layers' sequential recurrences run CONCURRENTLY (L1 two chunks
    behind L0), so every elementwise ACT/DVE op covers both layers in one
    instruction ([128,64] tiles instead of 2x [128,32]) — halving the
    per-instruction overhead that dominated those engines.
  * Reservoir x-projection (x@Win) stays resident in PSUM: the recurrent
    r@Wres matmuls accumulate straight onto it (no identity re-injection,
    no SBUF evacuation).
  * Reservoir state is carried as q = r/0.3 with Wres/Wg_r/Wp pre-scaled by
    0.3 host-side, turning the leak update into a single DVE op.
  * h/q histories live in rotating fused ring buffers written directly by
    the final DVE op of each step (no scalar.copy per step).
  * x is transposed and all weights cast to fp16 on the host; matmul weights
    DMA directly from fp16 DRAM.
"""
import sys
sys.path.insert(0, '/opt/trn_rl_repo')

import numpy as np

import concourse.bass as bass
import concourse.bacc as bacc
import concourse.mybir as mybir
from concourse.tile import TileContext
from concourse.bass_utils import run_bass_kernel_spmd
from concourse.masks import make_identity

F32 = mybir.dt.float32
F16 = mybir.dt.float16

B, T, IN, H, R, OUT = 64, 512, 256, 512, 512, 10
NCORES = 8
BL = B // NCORES            # batch per core = 8
C = 32                      # chunk length (timesteps)
NCH = T // C                # 16 chunks per layer
LEAK = 0.3
P = 128
KH = H // P                 # 4 k/m tiles over H/R
NB = KH * BL                # 32: one layer's state width
FW = 2 * NB                 # 64: fused (both-layer) width
SLOTS = 2 * C               # 64-slot ring for h/q history
CB = BL * C                 # 256: proj matmul moving cols
CP = 8                      # reservoir psum sub-block (timesteps per bank)
AF = mybir.ActivationFunctionType
ALU = mybir.AluOpType

LINS = {0: IN, 1: H}


def build_program():
    nc = bacc.Bacc()

    xT = nc.declare_dram_parameter("xT", [IN, BL * T], F16, isOutput=False)
    w = {}
    for l, lin in LINS.items():
        p = f"L{l}_"
        for nm, shp in (
                ("Win", [lin, R]), ("Wres", [R, R]),
                ("Wz", [lin + H, H]), ("Wr", [lin + H, H]),
                ("Wc", [lin + H, H]), ("Wg", [lin + H + R, H]),
                ("Wp", [R, H])):
            w[p + nm] = nc.declare_dram_parameter(p + nm, shp, F16, isOutput=False)
        for nm in ("bz", "br", "bc", "bg", "bp"):
            w[p + nm] = nc.declare_dram_parameter(p + nm, [H], F32, isOutput=False)
    w["Wo1"] = nc.declare_dram_parameter("Wo1", [H, H], F16, isOutput=False)
    w["bo1"] = nc.declare_dram_parameter("bo1", [H], F32, isOutput=False)
    w["Wo2"] = nc.declare_dram_parameter("Wo2", [H, OUT], F16, isOutput=False)
    out_ext = nc.declare_dram_parameter("out", [OUT, BL], F32, isOutput=True)

    with TileContext(nc) as tc:
        with tc.tile_pool(name="persist", bufs=1) as PERS, \
             tc.tile_pool(name="wts", bufs=1) as WP, \
             tc.tile_pool(name="chunk", bufs=2) as CH, \
             tc.tile_pool(name="step", bufs=4) as ST, \
             tc.tile_pool(name="ps", bufs=1, space="PSUM") as PS:
            emit_all(nc, tc, PERS, WP, CH, ST, PS, xT, w, out_ext)

    nc.compile()
    return nc


def emit_all(nc, tc, PERS, WP, CH, ST, PS, xT, w, out_ext):
    identf = PERS.tile([P, P], F32, tag="identf")
    make_identity(nc, identf[:])
    ident16 = PERS.tile([P, P], F16, tag="ident16")
    nc.vector.tensor_copy(ident16[:], identf[:])

    BLT = BL * T
    xsb = PERS.tile([P, 2 * BLT], F16, tag="xsb")
    for c in range(NCH):
        for k in range(2):
            src = xT[k * P:(k + 1) * P, :].rearrange("p (b t) -> p b t", b=BL)
            dst = xsb[:, k * BLT:(k + 1) * BLT].rearrange("p (b t) -> p b t", b=BL)
            nc.sync.dma_start(out=dst[:, :, c * C:(c + 1) * C],
                              in_=src[:, :, c * C:(c + 1) * C])

    # h / q history rings: col = slot*FW + l*NB + k*BL + b
    hist = PERS.tile([P, SLOTS * FW], F16, tag="hist")
    rhist = PERS.tile([P, SLOTS * FW], F16, tag="rhist")
    nc.vector.memset(hist[:, (SLOTS - 1) * FW:], 0.0)
    nc.vector.memset(rhist[:, (SLOTS - 1) * FW:], 0.0)

    # ---- weights ----
    def load_w(src, row0, nk, nm, tag):
        t = WP.tile([P, nk * nm * P], F16, tag=tag, name=tag)
        for k in range(nk):
            nc.gpsimd.dma_start(
                out=t[:, k * nm * P:(k + 1) * nm * P],
                in_=src[row0 + k * P: row0 + (k + 1) * P, :])
        return t

    def load_bias(src, nm, tag):
        t = WP.tile([P, nm], F32, tag=tag, name=tag)
        nc.sync.dma_start(out=t[:], in_=src.rearrange("(m p) -> p m", p=P))
        return t

    W = {}
    for l, lin in LINS.items():
        pfx = f"L{l}_"
        KL = lin // P
        # prep-side first (prologue needs them earliest)
        W[l, "win"] = load_w(w[pfx + "Win"], 0, KL, KH, f"win{l}")
        W[l, "res"] = load_w(w[pfx + "Wres"], 0, KH, KH, f"res{l}")
        # zr x-part packed: m 0..3 = z, 4..7 = r
        t = WP.tile([P, KL * 8 * P], F16, tag=f"zrx{l}", name=f"zrx{l}")
        for k in range(KL):
            nc.gpsimd.dma_start(out=t[:, (k * 8) * P:(k * 8 + 4) * P],
                                in_=w[pfx + "Wz"][k * P:(k + 1) * P, :])
            nc.gpsimd.dma_start(out=t[:, (k * 8 + 4) * P:(k * 8 + 8) * P],
                                in_=w[pfx + "Wr"][k * P:(k + 1) * P, :])
        W[l, "wzr_x"] = t
        W[l, "wc_x"] = load_w(w[pfx + "Wc"], 0, KL, KH, f"wcx{l}")
        W[l, "wg_x"] = load_w(w[pfx + "Wg"], 0, KL, KH, f"wgx{l}")
        W[l, "wg_r"] = load_w(w[pfx + "Wg"], lin + H, KH, KH, f"wgr{l}")
        W[l, "wp"] = load_w(w[pfx + "Wp"], 0, KH, KH, f"wp{l}")
        # recurrent h-parts
        t = WP.tile([P, KH * 8 * P], F16, tag=f"zrh{l}", name=f"zrh{l}")
        for k in range(KH):
            nc.gpsimd.dma_start(out=t[:, (k * 8) * P:(k * 8 + 4) * P],
                                in_=w[pfx + "Wz"][lin + k * P:lin + (k + 1) * P, :])
            nc.gpsimd.dma_start(out=t[:, (k * 8 + 4) * P:(k * 8 + 8) * P],
                                in_=w[pfx + "Wr"][lin + k * P:lin + (k + 1) * P, :])
        W[l, "wzr_h"] = t
        W[l, "wc_h"] = load_w(w[pfx + "Wc"], lin, KH, KH, f"wch{l}")
        W[l, "wg_h"] = load_w(w[pfx + "Wg"], lin, KH, KH, f"wgh{l}")
        t = WP.tile([P, 8], F32, tag=f"bzr{l}", name=f"bzr{l}")
        nc.sync.dma_start(out=t[:, 0:4], in_=w[pfx + "bz"].rearrange("(m p) -> p m", p=P))
        nc.sync.dma_start(out=t[:, 4:8], in_=w[pfx + "br"].rearrange("(m p) -> p m", p=P))
        W[l, "bzr"] = t
        W[l, "bc"] = load_bias(w[pfx + "bc"], KH, f"bc{l}")
        W[l, "bg"] = load_bias(w[pfx + "bg"], KH, f"bg{l}")
        W[l, "bp"] = load_bias(w[pfx + "bp"], KH, f"bp{l}")

    # ---- PSUM tiles (8 banks, [P,512] f32 each) ----
    pj = [PS.tile([P, 512], F32, tag=f"pj{i}", name=f"pj{i}") for i in range(2)]
    zrb = [PS.tile([P, 512], F32, tag=f"zr{i}", name=f"zr{i}") for i in range(2)]
    cgb = [PS.tile([P, 512], F32, tag=f"cg{i}", name=f"cg{i}") for i in range(2)]
    rxb = [PS.tile([P, 512], F32, tag=f"rx{i}", name=f"rx{i}") for i in range(2)]

    # ---- x-part chunk tiles ----
    # zrx: col = gate*(FW*C) + (l*NB + m*BL + b)*C + tau
    # cx/gx/ptil: col = (l*NB + m*BL + b)*C + tau
    def new_chunk_tiles(f):
        return {
            'zrx': CH.tile([P, 2 * FW * C], F16, tag="zrx", name=f"zrx_{f}"),
            'cx': CH.tile([P, FW * C], F16, tag="cx", name=f"cx_{f}"),
            'gx': CH.tile([P, FW * C], F16, tag="gx", name=f"gx_{f}"),
            'ptil': CH.tile([P, FW * C], F16, tag="ptil", name=f"ptil_{f}"),
            # q chunk shadow in (l,k,b,tau) layout: prep matmuls moving-read
            # it with 32-long contiguous runs (the slot-major rings stream at
            # half rate). Written per res step by the otherwise-idle GpSimd.
            'qsh': CH.tile([P, FW * C], F16, tag="qsh", name=f"qsh_{f}"),
        }

    chunk_tiles = {}
    hshadows = {}  # f -> [P, NB*C] (k,b,tau) shadow of L0's h chunk f

    def xrhs_bt(k, c):
        """x rhs slice for chunk c, k-tile: free dims (b, t-window)."""
        a = xsb[:, k * BLT:(k + 1) * BLT].rearrange("p (b t) -> p b t", b=BL)
        return a[:, :, c * C:(c + 1) * C]

    def xrhs_tb(k, c, s):
        """x rhs for projA sub-block s: free dims (t(8), b)."""
        a = xsb[:, k * BLT:(k + 1) * BLT].rearrange("p (b t) -> p t b", b=BL)
        return a[:, c * C + s * CP:c * C + (s + 1) * CP, :]

    def hrhs_bt(c, k):
        """L1 prep rhs: L0 h chunk c from its shadow; free dims (b, tau)."""
        a = hshadows[c].rearrange("p (u t) -> p u t", t=C)
        return a[:, k * BL:(k + 1) * BL, :]  # [p, b, tau], runs of C

    def hrhs_tb(c, k, s):
        a = hshadows[c].rearrange("p (u t) -> p u t", t=C)
        a = a[:, k * BL:(k + 1) * BL, s * CP:(s + 1) * CP]
        return a.rearrange("p b t -> p t b")

    def qrhs_bt(ct, l, k):
        """projB g-r/p rhs: q chunk shadow; dims (b, tau), runs of C."""
        a = ct['qsh'].rearrange("p (u t) -> p u t", t=C)
        return a[:, l * NB + k * BL:l * NB + (k + 1) * BL, :]

    def state_sl(tile, slot, ls):
        """[P, FW] (or [P,NB] single-layer) slice of a ring tile at slot."""
        if len(ls) == 2:
            return tile[:, slot * FW:(slot + 1) * FW]
        l = ls[0]
        return tile[:, slot * FW + l * NB:slot * FW + (l + 1) * NB]

    def state_kb(tile, slot, l, k):
        o = slot * FW + l * NB + k * BL
        return tile[:, o:o + BL]

    # ---------------- prep generator ----------------
    def prep_gen(f):
        """Emit prep for fused chunk f+1's GRU: L0 chunk f+1 + L1 chunk f-1."""
        cA = f + 1 if f + 1 <= NCH - 1 else None   # L0 chunk
        cB = f - 1 if 0 <= f - 1 <= NCH - 1 else None  # L1 chunk
        if cA is None and cB is None:
            return
        ls = [l for l, cc in ((0, cA), (1, cB)) if cc is not None]
        cc_of = {0: cA, 1: cB}
        ct = new_chunk_tiles(f)
        chunk_tiles[f + 1] = ct
        qbase = ((f + 1) * C) % SLOTS

        def projA_block(s):
            rx = rxb[s % 2]
            for l in ls:
                cl = cc_of[l]
                KL = LINS[l] // P
                for m in range(KH):
                    # out cols tau'*FW + l*NB + m*BL + b ; dims (tau', b)
                    o = rx.rearrange("p (t u) -> p t u", u=FW)[
                        :, :, l * NB + m * BL:l * NB + (m + 1) * BL]
                    for k in range(KL):
                        rhs = (xrhs_tb(k, cl, s) if l == 0 else hrhs_tb(cl, k, s))
                        nc.tensor.matmul(
                            o, W[l, "win"][:, (k * KH + m) * P:(k * KH + m + 1) * P],
                            rhs, start=(k == 0 and m == 0 and l == ls[0]),
                            stop=(k == KL - 1))
                        yield

        def res_steps(s):
            rx = rxb[s % 2]
            for tp in range(CP):
                tau = s * CP + tp
                slot = (qbase + tau) % SLOTS
                prev = (slot - 1) % SLOTS
                for l in ls:
                    for m in range(KH):
                        o = rx[:, tp * FW + l * NB + m * BL:
                               tp * FW + l * NB + (m + 1) * BL]
                        for k in range(KH):
                            nc.tensor.matmul(
                                o, W[l, "res"][:, (k * KH + m) * P:(k * KH + m + 1) * P],
                                state_kb(rhist, prev, l, k),
                                start=False, stop=(k == KH - 1))
                tt = ST.tile([P, len(ls) * NB], F16, tag="rtt")
                if len(ls) == 2:
                    src = rx[:, tp * FW:(tp + 1) * FW]
                else:
                    l = ls[0]
                    src = rx[:, tp * FW + l * NB:tp * FW + (l + 1) * NB]
                nc.scalar.activation(tt[:], src, AF.Tanh)
                nc.vector.scalar_tensor_tensor(
                    state_sl(rhist, slot, ls), state_sl(rhist, prev, ls),
                    1.0 - LEAK, tt[:], ALU.mult, ALU.add)
                qv = ct['qsh'].rearrange("p (u t) -> p u t", t=C)
                if len(ls) == 2:
                    qv = qv[:, :, tau]
                else:
                    qv = qv[:, ls[0] * NB:(ls[0] + 1) * NB, tau]
                nc.gpsimd.tensor_copy(qv, state_sl(rhist, slot, ls))
                yield

        # ---- projB ----
        pjc = [0]

        def proj(l, wtile, nk, m, nm_stride, rhs_of_k, out_ap, act_fn, bias,
                 extra=None):
            ps = pj[pjc[0] % 2]
            pjc[0] += 1
            psv = ps[:, :CB]
            for k in range(nk):
                nc.tensor.matmul(
                    psv, wtile[:, (k * nm_stride + m) * P:(k * nm_stride + m + 1) * P],
                    rhs_of_k(k), start=(k == 0), stop=(extra is None and k == nk - 1))
                yield
            if extra is not None:
                wt2, nk2, rhs2 = extra
                for k in range(nk2):
                    nc.tensor.matmul(
                        psv, wt2[:, (k * KH + m) * P:(k * KH + m + 1) * P],
                        rhs2(k), start=False, stop=(k == nk2 - 1))
                    yield
            nc.scalar.activation(out_ap, psv, act_fn, bias=bias)
            yield

        def xr_of(l):
            cl = cc_of[l]
            return (lambda k, _l=l, _c=cl: xrhs_bt(k, _c) if _l == 0
                    else hrhs_bt(_c, k))

        def zr_c_pieces():
            for l in ls:
                KL = LINS[l] // P
                xr = xr_of(l)
                # zr: gates z (g=0) / r (g=1), psum (b, tau)
                for mi in range(8):
                    g, m = mi // 4, mi % 4
                    o = ct['zrx'].rearrange("p (g u t) -> p g u t", g=2, t=C)[
                        :, g, l * NB + m * BL:l * NB + (m + 1) * BL, :]
                    yield from proj(l, W[l, "wzr_x"], KL, mi, 8, xr, o,
                                    AF.Identity, W[l, "bzr"][:, mi:mi + 1])
                for m in range(KH):
                    o = ct['cx'].rearrange("p (u t) -> p u t", t=C)[
                        :, l * NB + m * BL:l * NB + (m + 1) * BL, :]
                    yield from proj(l, W[l, "wc_x"], KL, m, KH, xr, o,
                                    AF.Identity, W[l, "bc"][:, m:m + 1])

        def gp_pieces():
            for l in ls:
                KL = LINS[l] // P
                xr = xr_of(l)
                for m in range(KH):
                    o = ct['gx'].rearrange("p (u t) -> p u t", t=C)[
                        :, l * NB + m * BL:l * NB + (m + 1) * BL, :]
                    yield from proj(l, W[l, "wg_x"], KL, m, KH, xr, o,
                                    AF.Identity, W[l, "bg"][:, m:m + 1],
                                    extra=(W[l, "wg_r"], KH,
                                           lambda k, _l=l: qrhs_bt(ct, _l, k)))
                for m in range(KH):
                    o = ct['ptil'].rearrange("p (u t) -> p u t", t=C)[
                        :, l * NB + m * BL:l * NB + (m + 1) * BL, :]
                    yield from proj(l, W[l, "wp"], KH, m, KH,
                                    lambda k, _l=l: qrhs_bt(ct, _l, k), o,
                                    AF.Tanh, W[l, "bp"][:, m:m + 1])

        # schedule: projA 0,1 up front; res steps interleaved with the
        # x-only zr/c projections (fillers that never wait on the res
        # chain); projA 2/3 after their banks are consumed; q-dependent
        # g/p projections last.
        fill = zr_c_pieces()
        yield from projA_block(0)
        yield from projA_block(1)
        for s in range(4):
            for _ in res_steps(s):
                yield
                for _ in range(2):
                    try:
                        next(fill)
                    except StopIteration:
                        break
                    else:
                        yield
            if s == 0:
                yield from projA_block(2)
            if s == 1:
                yield from projA_block(3)
        for _ in fill:
            yield
        yield from gp_pieces()

    def gen_pieces(f):
        cA = 1 if f + 1 <= NCH - 1 else 0
        cB = 1 if 0 <= f - 1 <= NCH - 1 else 0
        nA = cA * (KH * 2 * 4 + 8 * 3 + 4 * 3 + 4 * 7 + 4 * 5)
        nB = cB * (KH * 4 * 4 + 8 * 5 + 4 * 5 + 4 * 9 + 4 * 5)
        nres = C if (cA or cB) else 0
        return nA + nB + nres

    # ---------------- GRU ----------------
    def xpart_tau(ct, name, tau, ls):
        """rhs AP for the per-step x-part injection."""
        if name == 'zrx':
            a = ct['zrx'].rearrange("p (g u t) -> p g u t", g=2, t=C)
            if len(ls) == 2:
                return a[:, :, :, tau]
            l = ls[0]
            return a[:, :, l * NB:(l + 1) * NB, tau]
        a = ct[name].rearrange("p (u t) -> p u t", t=C)
        if len(ls) == 2:
            return a[:, :, tau]
        l = ls[0]
        return a[:, l * NB:(l + 1) * NB, tau]

    def gru_step(f, tau, adv):
        a0 = f <= NCH - 1
        a1 = 2 <= f <= NCH + 1
        ls = [l for l, a in ((0, a0), (1, a1)) if a]
        ct = chunk_tiles[f]
        s = (f * C + tau) % SLOTS
        sp = (s - 1) % SLOTS
        nls = len(ls)
        SW = nls * NB
        hprev = state_sl(hist, sp, ls)

        def gate_cols(ps, base, l, m):
            if nls == 2:
                o = base + l * NB + m * BL
            else:
                o = base + m * BL
            return ps[:, o:o + BL]

        zr = zrb[tau % 2]
        # inject x-parts for z and r (one matmul)
        zr_rhs = xpart_tau(ct, 'zrx', tau, ls)
        zr_out = zr[:, 0:128].rearrange("p (g u) -> p g u", g=2)[:, :, :SW]
        nc.tensor.matmul(zr_out, ident16[:], zr_rhs, start=True, stop=False)
        # r-gate matmuls first
        for l in ls:
            for m in range(KH):
                o = gate_cols(zr, 64, l, m)
                for k in range(KH):
                    nc.tensor.matmul(
                        o, W[l, "wzr_h"][:, (k * 8 + 4 + m) * P:(k * 8 + 5 + m) * P],
                        state_kb(hist, sp, l, k), start=False, stop=(k == KH - 1))
        reset = ST.tile([P, SW], F16, tag="reset")
        nc.scalar.activation(reset[:], zr[:, 64:64 + SW], AF.Sigmoid)
        for l in ls:
            for m in range(KH):
                o = gate_cols(zr, 0, l, m)
                for k in range(KH):
                    nc.tensor.matmul(
                        o, W[l, "wzr_h"][:, (k * 8 + m) * P:(k * 8 + m + 1) * P],
                        state_kb(hist, sp, l, k), start=False, stop=(k == KH - 1))
        rh = ST.tile([P, SW], F16, tag="rh")
        nc.vector.tensor_tensor(rh[:], reset[:], hprev, ALU.mult)
        zz = ST.tile([P, SW], F16, tag="zz")
        nc.scalar.activation(zz[:], zr[:, 0:SW], AF.Sigmoid)
        adv()

        cg = cgb[tau % 2]
        nc.tensor.matmul(cg[:, 0:SW], ident16[:], xpart_tau(ct, 'cx', tau, ls),
                         start=True, stop=False)
        for l in ls:
            for m in range(KH):
                o = gate_cols(cg, 0, l, m)
                for k in range(KH):
                    li = 0 if nls == 2 else None
                    rk = rh[:, (l * NB if nls == 2 else 0) + k * BL:
                            (l * NB if nls == 2 else 0) + (k + 1) * BL]
                    nc.tensor.matmul(
                        o, W[l, "wc_h"][:, (k * KH + m) * P:(k * KH + m + 1) * P],
                        rk, start=False, stop=(k == KH - 1))
        zh = ST.tile([P, SW], F16, tag="zh")
        nc.vector.tensor_tensor(zh[:], zz[:], hprev, ALU.mult)
        s1 = ST.tile([P, SW], F16, tag="s1")
        nc.vector.tensor_tensor(s1[:], hprev, zh[:], ALU.subtract)
        cc = ST.tile([P, SW], F16, tag="cc")
        nc.scalar.activation(cc[:], cg[:, 0:SW], AF.Tanh)
        zc = ST.tile([P, SW], F16, tag="zc")
        nc.vector.tensor_tensor(zc[:], zz[:], cc[:], ALU.mult)
        hg = ST.tile([P, SW], F16, tag="hg")
        nc.vector.tensor_tensor(hg[:], s1[:], zc[:], ALU.add)
        adv()

        nc.tensor.matmul(cg[:, 64:64 + SW], ident16[:],
                         xpart_tau(ct, 'gx', tau, ls), start=True, stop=False)
        for l in ls:
            for m in range(KH):
                o = gate_cols(cg, 64, l, m)
                for k in range(KH):
                    hk = hg[:, (l * NB if nls == 2 else 0) + k * BL:
                            (l * NB if nls == 2 else 0) + (k + 1) * BL]
                    nc.tensor.matmul(
                        o, W[l, "wg_h"][:, (k * KH + m) * P:(k * KH + m + 1) * P],
                        hk, start=False, stop=(k == KH - 1))
        d2 = ST.tile([P, SW], F16, tag="d2")
        nc.vector.tensor_tensor(d2[:], xpart_tau(ct, 'ptil', tau, ls), hg[:],
                                ALU.subtract)
        gg = ST.tile([P, SW], F16, tag="gg")
        nc.scalar.activation(gg[:], cg[:, 64:64 + SW], AF.Sigmoid)
        e2 = ST.tile([P, SW], F16, tag="e2")
        nc.vector.tensor_tensor(e2[:], gg[:], d2[:], ALU.mult)
        nc.vector.tensor_tensor(state_sl(hist, s, ls), hg[:], e2[:], ALU.add)
        if a0:
            hv = hshadows[f].rearrange("p (u t) -> p u t", t=C)[:, :, tau]
            nc.gpsimd.tensor_copy(hv, hist[:, s * FW:s * FW + NB])
        adv()

    # ---------------- main pipeline ----------------
    for _ in (prep_gen(-1) or ()):
        pass

    for f in range(NCH + 2):
        if f <= NCH - 1:
            hshadows[f] = CH.tile([P, NB * C], F16, tag="hsh", name=f"hsh_{f}")
        g = prep_gen(f)
        n_pieces = gen_pieces(f)
        box = {'gen': g, 'deficit': 0.0}
        per_point = n_pieces / (3.0 * C)

        def adv():
            if box['gen'] is None:
                return
            box['deficit'] += per_point
            while box['deficit'] >= 1.0:
                try:
                    next(box['gen'])
                except StopIteration:
                    box['gen'] = None
                    return
                box['deficit'] -= 1.0

        for tau in range(C):
            gru_step(f, tau, adv)
        while box['gen'] is not None:
            try:
                next(box['gen'])
            except StopIteration:
                box['gen'] = None
        if f - 1 in chunk_tiles:
            del chunk_tiles[f - 1]

    # ---------------- head ----------------
    wo1 = WP.tile([P, KH * KH * P], F16, tag="wo1")
    for k in range(KH):
        nc.gpsimd.dma_start(out=wo1[:, k * KH * P:(k + 1) * KH * P],
                            in_=w["Wo1"][k * P:(k + 1) * P, :])
    bo1 = WP.tile([P, KH], F32, tag="bo1")
    nc.sync.dma_start(out=bo1[:], in_=w["bo1"].rearrange("(m p) -> p m", p=P))
    wo2 = WP.tile([P, KH * OUT], F16, tag="wo2")
    for k in range(KH):
        nc.gpsimd.dma_start(out=wo2[:, k * OUT:(k + 1) * OUT],
                            in_=w["Wo2"][k * P:(k + 1) * P, :])

    # final L1 h: written at fused chunk NCH+1, tau=C-1 -> slot SLOTS-1, l=1
    fslot = ((NCH + 1) * C + C - 1) % SLOTS

    def hfin_k(k):
        return state_kb(hist, fslot, 1, k)

    o1 = PERS.tile([P, KH * BL], F16, tag="o1")
    ps1 = pj[0]
    for m in range(KH):
        for k in range(KH):
            nc.tensor.matmul(ps1[:, m * BL:(m + 1) * BL],
                             wo1[:, (k * KH + m) * P:(k * KH + m + 1) * P],
                             hfin_k(k), start=(k == 0), stop=(k == KH - 1))
    for m in range(KH):
        nc.scalar.activation(o1[:, m * BL:(m + 1) * BL],
                             ps1[:, m * BL:(m + 1) * BL],
                             AF.Relu, bias=bo1[:, m:m + 1])
    ps2 = pj[1]
    for k in range(KH):
        nc.tensor.matmul(ps2[:OUT, :BL], wo2[:, k * OUT:(k + 1) * OUT],
                         o1[:, k * BL:(k + 1) * BL],
                         start=(k == 0), stop=(k == KH - 1))
    osb = PERS.tile([OUT, BL], F32, tag="osb")
    nc.vector.tensor_copy(osb[:], ps2[:OUT, :BL])  # bo2 added host-side
    nc.sync.dma_start(out=out_ext[:], in_=osb[:])


_CACHED = {}


def _get_program():
    if "nc" not in _CACHED:
        _CACHED["nc"] = build_program()
    return _CACHED["nc"]


def _prep_weights(inputs):
    """Host-side: cast to fp16, pre-scale reservoir-consuming weights."""
    f = {}
    for l, lin in LINS.items():
        p = f"L{l}_"
        f[p + "Win"] = inputs[p + "Win"].astype(np.float16)
        f[p + "Wres"] = (inputs[p + "Wres"] * LEAK).astype(np.float16)
        f[p + "Wz"] = inputs[p + "Wz"].astype(np.float16)
        f[p + "Wr"] = inputs[p + "Wr"].astype(np.float16)
        f[p + "Wc"] = inputs[p + "Wc"].astype(np.float16)
        wg = inputs[p + "Wg"].copy()
        wg[lin + H:] *= LEAK
        f[p + "Wg"] = wg.astype(np.float16)
        f[p + "Wp"] = (inputs[p + "Wp"] * LEAK).astype(np.float16)
        for nm in ("bz", "br", "bc", "bg", "bp"):
            f[p + nm] = inputs[p + nm].astype(np.float32)
    f["Wo1"] = inputs["Wo1"].astype(np.float16)
    f["bo1"] = inputs["bo1"].astype(np.float32)
    f["Wo2"] = inputs["Wo2"].astype(np.float16)
    return f


def make_in_maps(inputs):
    inputs = {k: np.asarray(v) for k, v in inputs.items()}
    shared = _prep_weights(inputs)
    shared = {k: np.ascontiguousarray(v) for k, v in shared.items()}
    x = np.asarray(inputs["x"], dtype=np.float32)
    in_maps = []
    for i in range(NCORES):
        xc = x[i * BL:(i + 1) * BL]                     # [BL, T, IN]
        xt = np.ascontiguousarray(
            xc.transpose(2, 0, 1).reshape(IN, BL * T).astype(np.float16))
        m = {"xT": xt}
        m.update(shared)
        in_maps.append(m)
    return in_maps


def kernel(**inputs):
    nc = _get_program()
    in_maps = make_in_maps(inputs)
    res = run_bass_kernel_spmd(nc, in_maps, list(range(NCORES)))
    outs = [res.results[i]["out"].T for i in range(NCORES)]  # (BL, OUT)
    full = np.concatenate(outs, axis=0).astype(np.float32)
    return full + np.asarray(inputs["bo2"], dtype=np.float32)[None, :]


if __name__ == "__main__":
    nc = build_program()
    print("built program OK")
